# revision 45
# baseline (speedup 1.0000x reference)
"""Multi-head causal attention (B=2, S=2048, E=1024, H=16) on 8 TRN2 cores.

Sharding: 2-way data parallel on batch x 4-way tensor parallel on heads.
Core c handles batch b = c//4 and heads [4g, 4g+4) where g = c%4.
Each core computes q/k/v projections for its 4 heads, causal attention,
and a partial output projection (row-parallel Wo slice); the host sums
the 4 partials per batch and adds bo.

v6 design (~166 us at nominal clock; the device shows ~2-5 us
run-to-run clock-throttle noise, so all deltas were paired-A/B tested):
- Matmul operands are bf16, except Wq/Wk which ride as fp8 e3m4
  (host pre-scaled x64 to clear e3m4's subnormal floor, rescaled by
  folding 2^-12 into the exp's input scale). Mixed bf16-moving x
  fp8-stationary runs at the same 1 cycle/row PE rate and halves the
  front-critical weight DMA bytes. PSUM accumulation stays f32.
  (Wv in fp8 also works, with the denominator ones-rows set to 64 so
  the softmax ratio cancels the scale, but costs rel_err 7.4e-3 ->
  1.39e-2 for ~1 us: not taken; knob w8v.)
- The x stream moves in two 1024-column halves per 128-row e-chunk
  across the sync/scalar DGE rings (which share one ~260 GB/s hardware
  queue) with half 0 first, so tile-0/1 projections and v chunks 0-7
  unblock before the bulk of x lands. DMA instruction count stays ~25:
  more issues than the ~20 DGE semaphores stalls on sem recycling.
- Phase 1 is the minimal critical set for section (0,0): pair-0 q/k of
  tile 0, bvb, v chunks 0-1; everything else (pair-1 tile 0, v2-3, all
  later tiles) drips into the attention stream by deadline. Masked
  columns are skipped from d0>=128 (one extra diagonal chunk).
- Section-end softmax state is copied out of PSUM immediately (freeing
  the accumulation banks in ~2us) and the normalization multiplies are
  deferred into the filler stream, so neither the PSUM WAR hazard nor
  the broadcast-DMA latency ever stalls the PE or the DVE queue.
- The attention chunk loop is software-pipelined: QK^T + exp of chunk
  c+1 are emitted before P@V of chunk c, so the in-order PE never
  stalls on the Act-engine exp and holds its max p-state.
- Remaining projections and out-projection units are dripped into the
  PE stream at single-matmul granularity between attention chunks
  (deadline-scheduled, with a ready-delay for out-proj units and a
  small reserve kept for the tail).

Scores are computed transposed (k on partitions, q on free dim) so the
softmax denominator comes free as an extra ones-row in the P@V matmul,
and no P-tile transposes are needed anywhere.
"""

import sys

sys.path.insert(0, "/opt/trn_rl_repo")

from contextlib import ExitStack

import numpy as np

import concourse.bass as bass  # noqa: F401  (registers engines)
from concourse.ap import AP as _AP


def _free_bcast(src_ap, n):
    """View a [1, F] AP as [1, n, F] with a zero-stride middle dim (DMA replicate)."""
    return _AP(
        src_ap.tensor, src_ap.offset,
        [list(p) for p in src_ap.ap[:1]] + [[0, n]] + [list(p) for p in src_ap.ap[1:]],
    )

import concourse.tile as tile
from concourse import bacc, mybir
from concourse.bass_utils import run_bass_kernel_spmd

B, S, E, H = 2, 2048, 1024, 16
D = E // H            # 64
HPC = H // 4          # 4 heads per core
EC = HPC * D          # 256 = per-core head-dim width
NQT = S // 512        # 4 q-tiles of 512
NKC = S // 128        # 16 k-chunks of 128
NEC = E // 128        # 8 E-chunks of 128

F32 = mybir.dt.float32
BF16 = mybir.dt.bfloat16
EXP = mybir.ActivationFunctionType.Exp

# constants blob layout: [128, 961]
#   cols 0:896    staircase mask  M[kk, j] = 1.0 if j >= kk + 384 else 0
#   cols 896:898  ones, ones
#   cols 898:961  zeros
#   cols 961:963  64.0, 64.0  (denominator scale rows for fp8 wv)
#   cols 963:1026 zeros
# (the all-ones regions of the staircase double as ones-vectors:
#  row 0 is ones on cols [384:896))
CST_W = 1026

# v_sb per k-chunk: [128, 386]
#   h0: cols 0:64 v, 64 ones                 -> lhsT [0:65]   M=65  (sums row 64)
#   h1: col 65 ones, 66:129 zeros, 129:193 v -> lhsT [65:193] M=128 (sums row 0, data rows 64:128)
#   h2: cols 193:257 v, 257 ones             -> lhsT [193:258] M=65
#   h3: col 258 ones, 259:322 zeros, 322:386 v -> lhsT [258:386] M=128
V_W = 386
V_DATA = [0, 129, 193, 322]     # v data col start per local head
V_LHS = [(0, 65), (65, 193), (193, 258), (258, 386)]
V_STATIC = [64, 257]            # col starts of the [1,1,0*63] static blocks

# q/k tiles projected in phase 1 (in emission order); the rest are filler
PRE_QK = [(0, 0)]
N_WARM = 10           # zero matmuls to trip the HAM un-throttle before real work


def _build_nc(x_first=1024, oproj_split=False, n_warm=0,
              tail_rot=False, p1_zeros=True, qk4=False, pipe=1,
              rsv=(3, 12), w8=True, w8v=False, cst8=True,
              tail2q=True, psb8=False, tailz=12, offv=False,
              xbal=False, p1z2=False):
    nc = bacc.Bacc("TRN2", target_bir_lowering=False, debug=False, num_devices=8)

    WDT = mybir.dt.float8e3 if w8 else BF16
    xT = nc.dram_tensor("xT", [E, S], BF16, kind="ExternalInput")
    wq = nc.dram_tensor("wq", [E, EC], WDT, kind="ExternalInput")
    wk = nc.dram_tensor("wk", [E, EC], WDT, kind="ExternalInput")
    WVDT = mybir.dt.float8e3 if w8v else BF16
    wv = nc.dram_tensor("wv", [E, EC], WVDT, kind="ExternalInput")
    wo = nc.dram_tensor("wo", [EC, E], BF16, kind="ExternalInput")
    bqd = nc.dram_tensor("bq", [EC], F32, kind="ExternalInput")
    bkd = nc.dram_tensor("bk", [EC], F32, kind="ExternalInput")
    bvd = nc.dram_tensor("bv", [EC], BF16, kind="ExternalInput")
    CDT = mybir.dt.float8e3 if cst8 else BF16
    cst = nc.dram_tensor("cst", [128, CST_W], CDT, kind="ExternalInput")
    out = nc.dram_tensor("out", [S, E], BF16, kind="ExternalOutput")

    with tile.TileContext(nc) as tc:
        with ExitStack() as stack:
            cpool = stack.enter_context(tc.tile_pool(name="const", bufs=1))
            qkpool = stack.enter_context(tc.tile_pool(name="qkt", bufs=4))
            vpool = stack.enter_context(tc.tile_pool(name="vsb", bufs=NKC))
            wpool = stack.enter_context(tc.tile_pool(name="w", bufs=3))
            xpool = stack.enter_context(tc.tile_pool(name="xt", bufs=NEC))
            apool = stack.enter_context(tc.tile_pool(name="asb", bufs=2))
            ppool = stack.enter_context(tc.tile_pool(name="psb", bufs=3))
            rspool = stack.enter_context(tc.tile_pool(name="rs", bufs=2))
            bcpool = stack.enter_context(tc.tile_pool(name="bc", bufs=2))
            arpool = stack.enter_context(tc.tile_pool(name="ar", bufs=4))
            opool = stack.enter_context(tc.tile_pool(name="osb", bufs=4))

            # ---- PE warm-up: zero matmuls while the input DMAs stream ----
            # the HAM clock gate keeps the PE at 1.2 GHz until ~3.4us of
            # sustained busy; burn that window on zeros so every real
            # matmul runs at 2.4 GHz.
            warm_stack = ExitStack()
            warm_ps = warm_stack.enter_context(
                tc.tile_pool(name="warm_ps", bufs=2, space="PSUM"))
            zt = cpool.tile([128, 640], BF16, tag="zt")
            nc.vector.memset(zt[:], 0.0)

            def warm_mm():
                wp = warm_ps.tile([128, 512], F32, tag="warm", name="warmps")
                nc.tensor.matmul(
                    wp[:], zt[:, 0:128], zt[:, 128:640], start=True, stop=True)

            for i in range(n_warm):
                warm_mm()

            # preload the exp table set off the zero tile (no DMA dependency,
            # so it runs during the preamble instead of after the cst DMA)
            dummy = cpool.tile([1, 1], F32, tag="dummy")
            nc.scalar.activation(dummy[:], zt[0:1, 0:1], EXP)

            # ---- constants + weights + input DMAs ----
            # sync and scalar DGE rings share one ~260 GB/s hardware queue
            # (Q1); gpsimd (SWDGE) rides its own slower queue (Q10). Keep
            # the DMA count low (~25) — more issues than the ~20 DGE
            # semaphores stalls later issues on semaphore recycling. x moves
            # in two 1024-column halves per e-chunk (2 KB rows, near-full
            # packet efficiency), half 0 first: half 0 is everything tiles
            # 0-1 and v chunks 0-7 need, so attention starts ~5 us earlier.
            #   sync:   wq, x[e=0,3,6] h0, then h1
            #   scalar: cst, wk, x[e=1,4,7] h0, then h1
            #   gpsimd: biases, wv, x[e=2,5] h0, then h1, wo
            cst_sb = cpool.tile([128, CST_W], CDT, tag="cst")
            nc.scalar.dma_start(cst_sb[:], cst[:])
            static_blk = (cst_sb[:, 961:1026] if w8v
                          else cst_sb[:, 896:961])  # [128,65] = [s,s,0*63]
            ones_row0 = cst_sb[0:1, 384:512]       # [1,128] ones at partition 0

            bv_sb = cpool.tile([1, EC], BF16, tag="bv")
            nc.gpsimd.dma_start(bv_sb[:], bvd.ap().rearrange("(o n) -> o n", o=1))
            bq_sb = cpool.tile([128, 2], F32, tag="bq")
            nc.gpsimd.dma_start(bq_sb[:], bqd.ap().rearrange("(b p) -> p b", p=128))
            bk_sb = cpool.tile([128, 2], F32, tag="bk")
            nc.gpsimd.dma_start(bk_sb[:], bkd.ap().rearrange("(b p) -> p b", p=128))

            w_sb = {}
            w_eng = {"q": nc.sync, "k": nc.scalar, "v": nc.gpsimd}
            w_dt = {"q": WDT, "k": WDT, "v": WVDT}
            w_dram = {"q": wq, "k": wk, "v": wv}

            def w_dma(name):
                t = wpool.tile([128, NEC * EC], w_dt[name], tag=f"w{name}",
                               name=f"w{name}")
                w_eng[name].dma_start(
                    t[:].rearrange("p (e n) -> p e n", e=NEC),
                    w_dram[name].ap().rearrange("(e p) n -> p e n", p=128),
                )
                w_sb[name] = t

            for name in ("q", "k"):
                w_dma(name)
            if not xbal:
                w_dma("v")

            def w_lhs(name, e, pb):
                base = e * EC + pb * 128
                return w_sb[name][:, base:base + 128]

            x_eng = [nc.sync, nc.scalar, nc.gpsimd, nc.sync, nc.scalar,
                     nc.gpsimd, nc.sync, nc.scalar]
            xt_sb = [xpool.tile([128, S], BF16, tag="xt", name=f"xt{e}")
                     for e in range(NEC)]
            wo_sb = [cpool.tile([128, E], BF16, tag=f"wo{j}", name=f"wo{j}")
                     for j in range(2)]
            # first half (tiles 0-1 + v chunks 0-7) ahead of the bulk, so
            # attention unblocks on ~3 MB of critical input, not 4.75 MB
            def x_dma(e, lo, hi, eng=None):
                (eng or x_eng[e]).dma_start(
                    xt_sb[e][:, lo:hi], xT[e * 128:(e + 1) * 128, lo:hi])

            if xbal:
                # equalize queue TIME on the critical half: Q10 (~120 GB/s)
                # takes 3 of 8 h0 blocks, Q1 (~260, shared sync+scalar) the
                # rest; wv threads between so v chunks still unblock in time
                for e in (2, 5):
                    x_dma(e, 0, x_first)
                w_dma("v")
                for e in (0, 1, 3, 4, 6):
                    x_dma(e, 0, x_first)
                x_dma(7, 0, x_first, eng=nc.gpsimd)
                x_ranges = [(x_first, S)]
            else:
                x_ranges = [(0, x_first)] + (
                    [(x_first, S)] if x_first < S else [])
            for lo, hi in x_ranges:
                for e in range(NEC):
                    x_dma(e, lo, hi)
                if lo:
                    for j in range(2):
                        nc.gpsimd.dma_start(
                            wo_sb[j][:], wo[j * 128:(j + 1) * 128, :])

            # ---- persistent SBUF destinations ----
            # pair p rows: head 2p at partitions 0:64, head 2p+1 at 64:128
            qt_sb = [qkpool.tile([128, S], BF16, tag="qkt", name=f"qt{i}") for i in range(2)]
            kt_sb = [qkpool.tile([128, S], BF16, tag="qkt", name=f"kt{i}") for i in range(2)]
            v_sb = [vpool.tile([128, V_W], BF16, tag="vsb", name=f"v{m}") for m in range(NKC)]
            a_sb = [apool.tile([128, S], BF16, tag="asb", name=f"a{i}") for i in range(2)]
            bvb_sb = cpool.tile([128, EC], F32, tag="bvb")

            # ---- projection / out-projection generators ----
            # each yields after every matmul so the dripper can interleave at
            # single-matmul granularity
            def gen_qk_tile(name, dst, bias, pb, t, pool, tag):
                ps = pool.tile([128, 512], F32, tag=tag, name="qkps")
                for e in range(NEC):
                    nc.tensor.matmul(
                        ps[:],
                        w_lhs(name, e, pb),
                        xt_sb[e][:, t * 512:(t + 1) * 512],
                        start=(e == 0),
                        stop=(e == NEC - 1),
                    )
                    if e < NEC - 1:
                        yield
                nc.vector.tensor_scalar_add(
                    dst[pb][:, t * 512:(t + 1) * 512], ps[:], bias[:, pb:pb + 1]
                )
                yield

            def gen_v_chunk(m, pool, tag):
                vt = v_sb[m]
                for colstart in V_STATIC:
                    nc.gpsimd.tensor_copy(vt[:, colstart:colstart + 65], static_blk)
                vps = pool.tile([128, 512], F32, tag=tag, name="vps")
                for e in range(NEC):
                    nc.tensor.matmul(
                        vps[:, 0:EC],
                        xt_sb[e][:, m * 128:(m + 1) * 128],
                        w_sb["v"][:, e * EC:(e + 1) * EC],
                        start=(e == 0),
                        stop=(e == NEC - 1),
                    )
                    if e < NEC - 1:
                        yield
                for h in range(HPC):
                    d0 = V_DATA[h]
                    nc.vector.tensor_add(
                        vt[:, d0:d0 + 64],
                        vps[:, h * 64:(h + 1) * 64],
                        bvb_sb[:, h * 64:(h + 1) * 64],
                    )
                yield

            def gen_oproj(m, n, pool, tag, width=512, dma_eng=None):
                # out-proj unit: out[q,e] = sum_hd A[hd,q] Wo[hd,e]
                # (width=256 for the last tile: a finer tail chain)
                ops = pool.tile([128, 512], F32, tag=tag, name="ops")
                nc.tensor.matmul(
                    ops[:, 0:width],
                    a_sb[0][:, m * 128:(m + 1) * 128],
                    wo_sb[0][:, n * width:(n + 1) * width],
                    start=True, stop=False,
                )
                yield
                nc.tensor.matmul(
                    ops[:, 0:width],
                    a_sb[1][:, m * 128:(m + 1) * 128],
                    wo_sb[1][:, n * width:(n + 1) * width],
                    start=False, stop=True,
                )
                osb = opool.tile([128, 512], BF16, tag="osb", name="osb")
                (nc.vector.tensor_copy if (m + n) % 2 == 0 else nc.scalar.copy)(
                    osb[:, 0:width], ops[:, 0:width])
                if dma_eng is None:
                    dma_eng = nc.sync if n == 0 else nc.gpsimd
                dma_eng.dma_start(
                    out[m * 128:(m + 1) * 128, n * width:(n + 1) * width],
                    osb[:, 0:width],
                )
                yield

            # ---- phase 1: pair-0 q/k of tile 0 + bvb + v chunks 0-1 ----
            # (everything else is dripped into the attention stream; this is
            # the minimal critical set for section (0,0) to start.)
            # A zero matmul is interleaved every other step: the phase is
            # paced by the arriving x stream, and the zeros both absorb the
            # trickle stalls and keep the HAM clock gate at 8/8.
            pre_stack = ExitStack()
            pre_ps = pre_stack.enter_context(
                tc.tile_pool(name="pre_ps", bufs=4, space="PSUM"))

            # accumulators interleaved against the arriving x stream
            pre_gens = []
            for (pb, t) in PRE_QK:
                pre_gens.append(gen_qk_tile("q", qt_sb, bq_sb, pb, t, pre_ps, "pre"))
                pre_gens.append(gen_qk_tile("k", kt_sb, bk_sb, pb, t, pre_ps, "pre"))
            live = list(pre_gens)
            rounds = 0
            while live:
                if p1_zeros and rounds % 2 == 0:
                    warm_mm()
                if p1z2 and rounds >= 5:
                    # the e5-e7 x blocks cannot land before ~16us; the PE
                    # stalls here >3.4us and the HAM clock gate re-throttles,
                    # making early attention run at 1.2 GHz. Dense zeros
                    # bridge the known stall and keep the gate at 8/8.
                    warm_mm()
                    warm_mm()
                rounds += 1
                live = [g for g in live if next(g, "done") != "done"]
            # bv broadcast [128, EC] = ones[1,128].T @ bv[1,EC] (emitted
            # after the q/k matmuls so a late bv DMA can't stall the PE)
            bvb_ps = pre_ps.tile([128, 512], F32, tag="pre", name="bvb_ps")
            nc.tensor.matmul(
                bvb_ps[:, 0:EC], ones_row0, bv_sb[:], start=True, stop=True
            )
            nc.vector.tensor_copy(bvb_sb[:], bvb_ps[:, 0:EC])
            for m in range(2):
                for k, _ in enumerate(gen_v_chunk(m, pre_ps, "pre")):
                    if p1_zeros and k % 2 == 0:
                        warm_mm()
            if p1z2:
                for _ in range(4):
                    warm_mm()
            pre_stack.close()
            warm_stack.close()

            # ---- phase 2: pipelined attention with dripped filler ----
            attn_stack = ExitStack()
            qk_ps = attn_stack.enter_context(
                tc.tile_pool(name="qk_ps", bufs=2, space="PSUM"))
            at_ps = attn_stack.enter_context(
                tc.tile_pool(name="at_ps", bufs=2, space="PSUM"))
            fill_ps = attn_stack.enter_context(
                tc.tile_pool(name="fill_ps", bufs=2, space="PSUM"))

            # flat job list: (pair, q-tile, chunk); chunk c covers k rows
            # [c*128, (c+1)*128) for q-tile t (q cols [t*512, (t+1)*512))
            jobs = []
            sec_start = {}
            for t in range(NQT):
                for p in range(2):
                    sec_start[(p, t)] = len(jobs)
                    for c in range(4 * (t + 1)):
                        jobs.append((p, t, c))
            njobs = len(jobs)

            # filler queue entries: [deadline, ready_after, generator]
            # deadline: all steps must be emitted before this job index runs;
            # ready_after: don't start stepping before this job index.
            filler = []
            for m in range(2, 4):
                # v chunk m is first read by the PV emitted at job m+1
                filler.append([m + 1, 0, gen_v_chunk(m, fill_ps, "fill")])
            for t in range(NQT):
                for p in range(2):
                    if (p, t) in PRE_QK:
                        continue
                    dl = sec_start[(p, t)]
                    filler.append([dl, 0, gen_qk_tile(
                        "q", qt_sb, bq_sb, p, t, fill_ps, "fill")])
                    filler.append([dl, 0, gen_qk_tile(
                        "k", kt_sb, bk_sb, p, t, fill_ps, "fill")])
                if t >= 1:
                    s0 = sec_start[(0, t)]
                    for m in range(4 * t, 4 * (t + 1)):
                        # chunk m first read at job s0 + m + 1 of section (0,t)
                        filler.append([s0 + m + 1, 0, gen_v_chunk(m, fill_ps, "fill")])

            def drip(idx, steps):
                # deadline-forced items first (complete them fully) — the
                # correctness backstop for the in-order PE stream
                for ent in [f for f in filler if f[0] <= idx + 1]:
                    g = ent[2]
                    while next(g, "done") != "done":
                        pass
                    filler.remove(ent)
                # spread upcoming-deadline work over the preceding jobs so
                # it never lands as one burst that delays the next QK
                if any(f[0] <= idx + 4 for f in filler):
                    steps = max(steps, 5)
                # tail reserve: keep units back so the PE has work in the
                # ACT-paced endgame and while the final norm chain resolves
                if len(filler) <= rsv[0] and idx < njobs - rsv[1]:
                    return
                while steps > 0 and filler:
                    ent = None
                    for cand in sorted(filler, key=lambda f: f[0]):
                        if cand[1] <= idx:
                            ent = cand
                            break
                    if ent is None:
                        return
                    if next(ent[2], "done") == "done":
                        filler.remove(ent)
                    steps -= 1

            # per-section live state
            sec_ps = {}    # (p, t) -> (ape, apo)
            sec_psb = {}   # job idx -> (psb, q0) for the pending PV

            def emit_qk_chunk(idx, p, t, c):
                d0 = c * 128 - t * 512
                # columns below d0 are fully masked: skip them on diagonal
                # chunks (d0>=128); the d0=0 chunk keeps one wide exp
                q0 = d0 if d0 >= 128 else 0
                qsl = slice(t * 512 + q0, (t + 1) * 512)
                qkp = qk_ps.tile([128, 1024], F32, tag="qk", name="qkp")
                # scoresT [k-chunk, q-tile], both heads row-packed
                if qk4:
                    # four 64x64 tiles at (row,col) grp (h*64, half*64):
                    # disjoint row AND col groups -> true tile concurrency
                    for hh in range(2):
                        rr = slice(64 * hh, 64 * hh + 64)
                        for hf in range(2):
                            nc.tensor.matmul(
                                qkp[64 * hf:64 * hf + 64,
                                    512 * hh + q0:512 * hh + 512],
                                kt_sb[p][rr, c * 128 + 64 * hf:
                                         c * 128 + 64 * hf + 64],
                                qt_sb[p][rr, qsl],
                                start=True, stop=True,
                            )
                else:
                    nc.tensor.matmul(
                        qkp[:, q0:512],
                        kt_sb[p][0:64, c * 128:(c + 1) * 128],
                        qt_sb[p][0:64, qsl],
                        start=True, stop=True,
                    )
                    nc.tensor.matmul(
                        qkp[:, 512 + q0:1024],
                        kt_sb[p][64:128, c * 128:(c + 1) * 128],
                        qt_sb[p][64:128, qsl],
                        start=True, stop=True,
                    )
                psb = ppool.tile([128, 1024],
                                 mybir.dt.float8e3 if psb8 else BF16,
                                 tag="psb", name="psb")
                esc = float(2.0 ** -12) if w8 else 1.0
                if q0 == 0:
                    nc.scalar.activation(psb[:], qkp[:], EXP, scale=esc)
                else:
                    nc.scalar.activation(
                        psb[:, q0:512], qkp[:, q0:512], EXP, scale=esc)
                    nc.scalar.activation(
                        psb[:, 512 + q0:1024], qkp[:, 512 + q0:1024], EXP,
                        scale=esc)
                if d0 >= 0:
                    off = 384 - d0
                    for hh in range(2):
                        eng = nc.gpsimd if (offv and hh == 1) else nc.vector
                        eng.tensor_mul(
                            psb[:, hh * 512 + q0:(hh + 1) * 512],
                            psb[:, hh * 512 + q0:(hh + 1) * 512],
                            cst_sb[:, off + q0:off + 512],
                        )
                sec_psb[idx] = (psb, q0)

            def emit_pv(idx, p, t, c):
                psb, q0 = sec_psb.pop(idx)
                ape, apo = sec_ps[(p, t)]
                lhs_e = V_LHS[2 * p]      # even head of the pair
                lhs_o = V_LHS[2 * p + 1]  # odd head
                nchunks = 4 * (t + 1)
                first, last = (c == 0), (c == nchunks - 1)
                nc.tensor.matmul(
                    ape[0:65, q0:512],
                    v_sb[c][:, lhs_e[0]:lhs_e[1]],
                    psb[:, q0:512],
                    start=first, stop=last,
                )
                nc.tensor.matmul(
                    apo[:, q0:512],
                    v_sb[c][:, lhs_o[0]:lhs_o[1]],
                    psb[:, 512 + q0:1024],
                    start=first, stop=last,
                )
                return last

            # [1,64] ones rows for the denominator broadcast matmuls; the
            # lhsT must share its base partition with the rhs row, so pull
            # ones from mask row 64 (ones at cols >= 448) and row 0
            ones64_p64 = cst_sb[64:65, 448:512]
            ones64_p0 = cst_sb[0:1, 384:448]

            def gen_finish_norm(p, t, are, aro, idx):
                # deferred normalization: broadcast the denominator rows
                # down 64 partitions with two tiny PE matmuls (no DMA on
                # the critical chain), then reciprocal + multiply on DVE
                bcp = fill_ps.tile([128, 512], F32, tag="fill", name="bcp")
                nc.tensor.matmul(
                    bcp[0:64, :], ones64_p64, are[64:65, :], start=True, stop=True)
                nc.tensor.matmul(
                    bcp[64:128, :], ones64_p0, aro[0:1, :], start=True, stop=True)
                yield
                rcp = bcpool.tile([128, 512], F32, tag="bc", name="rcp")
                nc.vector.reciprocal_approx_fast(out=rcp[:], in_=bcp[:])
                nc.vector.tensor_mul(
                    a_sb[p][0:64, t * 512:(t + 1) * 512],
                    are[0:64, :], rcp[0:64, :],
                )
                (nc.gpsimd if offv else nc.vector).tensor_mul(
                    a_sb[p][64:128, t * 512:(t + 1) * 512],
                    aro[64:128, :], rcp[64:128, :],
                )
                if p == 1:
                    # both pairs of q-tile t normalized -> out-proj ready;
                    # the last tile goes in 256-wide pieces across all three
                    # DMA queues so its serial tail chain is finer-grained
                    if oproj_split and t == NQT - 1:
                        dq = [nc.sync, nc.gpsimd, nc.scalar]
                        u = 0
                        for m in range(4 * t, 4 * (t + 1)):
                            for n in range(4):
                                filler.append(
                                    [njobs, idx + 2,
                                     gen_oproj(m, n, fill_ps, "fill",
                                               width=256, dma_eng=dq[u % 3])])
                                u += 1
                    elif (tail_rot or tail2q) and t == NQT - 1:
                        dq = ([nc.sync, nc.scalar] if tail2q
                              else [nc.sync, nc.gpsimd, nc.scalar])
                        u = 0
                        for m in range(4 * t, 4 * (t + 1)):
                            for n in range(2):
                                filler.append(
                                    [njobs, idx + 2,
                                     gen_oproj(m, n, fill_ps, "fill",
                                               dma_eng=dq[u % len(dq)])])
                                u += 1
                    else:
                        for m in range(4 * t, 4 * (t + 1)):
                            for n in range(2):
                                filler.append(
                                    [njobs, idx + 2,
                                     gen_oproj(m, n, fill_ps, "fill")])
                yield

            def emit_norm(idx, p, t):
                # copy the accumulators out of PSUM right away (frees the
                # banks for the next section); everything else is deferred
                ape, apo = sec_ps.pop((p, t))
                are = arpool.tile([128, 512], BF16, tag="ar", name="are")
                aro = arpool.tile([128, 512], BF16, tag="ar", name="aro")
                nc.vector.tensor_copy(are[0:65, :], ape[0:65, :])
                nc.scalar.copy(aro[:], apo[:])
                filler.insert(0, [idx + 6, idx + 2,
                                  gen_finish_norm(p, t, are, aro, idx)])

            pend = []
            for idx, (p, t, c) in enumerate(jobs):
                if c == 0:
                    ape = at_ps.tile([128, 512], F32, tag="at", name="ape")
                    apo = at_ps.tile([128, 512], F32, tag="at", name="apo")
                    sec_ps[(p, t)] = (ape, apo)
                emit_qk_chunk(idx, p, t, c)
                boundary = False
                if len(pend) >= pipe:
                    pj = pend.pop(0)
                    if emit_pv(*pj):
                        emit_norm(pj[0], pj[1], pj[2])
                        boundary = True
                drip(idx, 6 if boundary else 2)
                pend.append((idx, p, t, c))
            for pj in pend:
                if emit_pv(*pj):
                    emit_norm(pj[0], pj[1], pj[2])

            # trailing drain. With tailz, zero matmuls are woven in so the
            # PE array stays busy through the final norm chain's dependency
            # stalls — otherwise the HAM clock gate re-throttles and the
            # last tile's out-projection runs at 1.2 GHz instead of 2.4.
            def tail_zero():
                zp = qk_ps.tile([128, 1024], F32, tag="qk", name="qkp")
                nc.tensor.matmul(
                    zp[:, 0:512], zt[:, 0:128], zt[:, 128:640],
                    start=True, stop=True)

            for _ in range(min(tailz, 4)):
                tail_zero()
            k = 0
            while k < len(filler):   # gens may append more filler mid-drain
                for _ in filler[k][2]:
                    pass
                if k == 0:
                    for _ in range(max(0, tailz - 4)):
                        tail_zero()
                k += 1
            filler.clear()
            attn_stack.close()

    nc.compile()
    return nc


_NC = {}


def _get_nc(**opts):
    key = tuple(sorted(
        (k, tuple(v) if isinstance(v, list) else v) for k, v in opts.items()))
    if key not in _NC:
        _NC[key] = _build_nc(**opts)
    return _NC[key]


def _constants():
    from ml_dtypes import bfloat16
    kk = np.arange(128, dtype=np.int64)[:, None]
    jj = np.arange(896, dtype=np.int64)[None, :]
    cst = np.zeros((128, CST_W), dtype=np.float32)
    cst[:, 0:896] = (jj >= kk + 384).astype(np.float32)
    cst[:, 896] = 1.0
    cst[:, 897] = 1.0
    cst[:, 961] = 64.0
    cst[:, 962] = 64.0
    return cst.astype(bfloat16)


def _in_maps(inputs, Wq, bq, Wk, bk, Wv, bv, Wo, bo, w8=True, w8v=False,
             cst8=True):
    from ml_dtypes import bfloat16, float8_e3m4
    cst = _constants()
    if cst8:
        cst = cst.astype(float8_e3m4)
    scale = np.float32(1.0 / np.sqrt(D))
    wsc = np.float32(64.0) if w8 else np.float32(1.0)
    wdt = float8_e3m4 if w8 else bfloat16
    vsc = np.float32(64.0) if w8v else np.float32(1.0)
    vdt = float8_e3m4 if w8v else bfloat16
    xT = [np.ascontiguousarray(inputs[b].T).astype(bfloat16) for b in range(B)]

    in_maps = []
    for c in range(8):
        b, g = divmod(c, 4)
        sl = slice(g * EC, (g + 1) * EC)
        in_maps.append({
            "xT": xT[b],
            "wq": (np.ascontiguousarray(Wq[:, sl]) * scale * wsc).astype(wdt),
            "bq": (bq[sl] * scale * wsc).astype(np.float32),
            "wk": (np.ascontiguousarray(Wk[:, sl]) * wsc).astype(wdt),
            "bk": (bk[sl] * wsc).astype(np.float32),
            "wv": (np.ascontiguousarray(Wv[:, sl]) * vsc).astype(vdt),
            "bv": (bv[sl] * vsc).astype(bfloat16),
            "wo": np.ascontiguousarray(Wo[sl, :]).astype(bfloat16),
            "cst": cst,
        })
    return in_maps


def kernel(inputs, Wq, bq, Wk, bk, Wv, bv, Wo, bo):
    inputs = np.asarray(inputs, dtype=np.float32)
    Wq = np.asarray(Wq, dtype=np.float32)
    Wk = np.asarray(Wk, dtype=np.float32)
    Wv = np.asarray(Wv, dtype=np.float32)
    Wo = np.asarray(Wo, dtype=np.float32)
    bq = np.asarray(bq, dtype=np.float32)
    bk = np.asarray(bk, dtype=np.float32)
    bv = np.asarray(bv, dtype=np.float32)
    bo = np.asarray(bo, dtype=np.float32)

    nc = _get_nc()
    in_maps = _in_maps(inputs, Wq, bq, Wk, bk, Wv, bv, Wo, bo)
    res = run_bass_kernel_spmd(nc, in_maps, list(range(8)))
    outs = [np.asarray(r["out"], dtype=np.float32) for r in res.results]
    full = np.empty((B, S, E), dtype=np.float32)
    for b in range(B):
        full[b] = outs[4 * b] + outs[4 * b + 1] + outs[4 * b + 2] + outs[4 * b + 3]
        full[b] += bo
    return full

